# revision 14
# baseline (speedup 1.0000x reference)
"""GAT layer (nn_GATLayerAdj) Trainium2 Bass kernel, 8-core SPMD, v3.

Reference computation (N=1024, di=do=64):
    a[i,j]  = x[j]@w_src + x[i]@w_tgt + bw        (attention logits)
    att     = softmax_j(where(adj>0, a, -1e16))
    y[i,j,:]= relu(x[j]@WfS.T + x[i]@WfT.T + bf)
    o[i,:]  = sum_j att[i,j] * y[i,j,:]

Sharding: target dim i split across 8 cores (128 rows each); row-wise
softmax needs no cross-core communication.

v3 = v1's proven compute layout (DVE tensor_tensor broadcast adds from
a DMA-broadcast urep; bf16 col-banded PE reduce) plus:
  - transposed softmax: e^T[j,i] built directly via 3 accumulating PE
    matmuls per chunk (asrc col + atgt row + additive adj mask through
    an identity matmul) and one ACT exp straight to bf16. Kills v1's 8
    PE transposes, 8 DVE psum->sbuf copies and the DVE adj multiply.
  - 1/s normalization and the 32-wide diagonal gather moved to the
    host: t_acc ships raw as bf16 (512KB) + row sums s (4KB). Kills
    the scaled fp32 evac and halves the tail DMA.
  - relu split rebalanced: DVE tensor_scalar 4x takes 5/16, ACT 11/16,
    balancing DVE (adds-heavy) against ACT (relu-heavy).

Per-core budget estimate: DVE ~41us (adds 34 + relu 5 + misc), ACT
~40us (relu 37 + exp 2.4), PE ~36us (reduce 27 + ldweights + logits).
An all-PE fp8-DoubleRow add variant was tried and is 2x WORSE: every
matmul self-reloads weights (--enable-ldw-opt=false), taxing the PE
~150ns per 512-col matmul, and DoubleRow rejects tile_position != 0
so the banded reduce can't use it.
"""

from contextlib import ExitStack

import numpy as np
import ml_dtypes

import concourse.bass as bass
import concourse.tile as tile
from concourse import bacc, mybir
from concourse.bass_utils import run_bass_kernel_spmd

# Lighter TileContext exit: stock emits drain + full butterfly barrier +
# sem clears + second butterfly (~11us). Engines already sync at program
# end; keep the drain, a sem-only rendezvous, drop the trailing barrier.
import concourse.tile as _tile_mod

if not getattr(_tile_mod, "_exit_trimmed", False):
    def _drain_and_barrier_trim(self, tick_clock, wait_clock):
        from concourse.tile import ScopedClock
        nc = self.nc
        drain_inst = nc.sync.drain()
        wait_clock.add_sem_waits(
            drain_inst.ins, ScopedClock({None: tick_clock.global_clock})
        )
        exit_sem = nc.alloc_semaphore("exit_rdv")
        for eng in (nc.sync, nc.tensor, nc.vector, nc.scalar):
            eng.nop(nofuse=True).then_inc(exit_sem, 1)
        nc.gpsimd.wait_ge(exit_sem, 4)
        assert self.sems is not None
        popped = nc._tile_sem_poison_stack.pop()
        assert popped is self._sem_poison
        nc.clear_and_free_semaphores(list(self.sems.allocated().values()))
        nc.gpsimd.sem_clear(range(exit_sem.num, exit_sem.num + 1))

    _tile_mod.TileContext._drain_and_barrier = _drain_and_barrier_trim
    _tile_mod._exit_trimmed = True

N = 1024
DI = 64
DO = 64
N_CORES = 8
ROWS = N // N_CORES          # 128 target rows per core
NCHUNK = N // 128            # 8 j-chunks
F_FULL = ROWS * DO           # 8192 free size of (i, d)
HALF = F_FULL // 2           # 4096: half-chunk unit

f32 = mybir.dt.float32
bf16 = mybir.dt.bfloat16
AF = mybir.ActivationFunctionType
ALU = mybir.AluOpType

BF = ml_dtypes.bfloat16

# unit index u = 2*c + h (16 units of [128, 4096]); engine assignment.
# ACT takes 11/16 of the relus, DVE (tensor_scalar 4x) 5/16.
RELU_ACT_UNITS = {0, 1, 2, 4, 5, 6, 8, 10, 14}
# chunk whose adds run on GPSIMD (Pool) to offload DVE; must only be
# needed late (its reduce is last)
POOL_ADD_CHUNKS = {7}

_CACHE = {}


def _build_program():
    nc = bacc.Bacc("TRN2", target_bir_lowering=False, debug=False,
                   num_devices=N_CORES)

    # ---- DRAM I/O ----
    xT_d = nc.dram_tensor("xT", [DI, N], bf16, kind="ExternalInput").ap()
    wfsT_d = nc.dram_tensor("wfsT", [DI, DO], bf16, kind="ExternalInput").ap()
    ws_d = nc.dram_tensor("ws", [DI, 1], bf16, kind="ExternalInput").ap()
    wta_d = nc.dram_tensor("wta", [DI + 1, 1], bf16, kind="ExternalInput").ap()
    wfta_d = nc.dram_tensor("wfta", [DI + 1, DO], bf16, kind="ExternalInput").ap()
    xbTa_d = nc.dram_tensor("xbTa", [DI + 1, ROWS], bf16, kind="ExternalInput").ap()
    adjmT_d = nc.dram_tensor("adjmT", [N, ROWS], bf16, kind="ExternalInput").ap()
    ident_d = nc.dram_tensor("ident", [128, 128], bf16, kind="ExternalInput").ap()
    ones_d = nc.dram_tensor("onesrow", [1, ROWS], bf16, kind="ExternalInput").ap()
    s_d = nc.dram_tensor("s8", [1, N], f32, kind="ExternalOutput").ap()
    o_d = nc.dram_tensor("o", [128, 2048], bf16, kind="ExternalOutput").ap()

    with tile.TileContext(nc) as tc, ExitStack() as ctx:
        cons = ctx.enter_context(tc.tile_pool(name="cons", bufs=1))
        zp = ctx.enter_context(tc.tile_pool(name="zp", bufs=4))
        rp = ctx.enter_context(tc.tile_pool(name="rp", bufs=3))
        psp = ctx.enter_context(tc.tile_pool(name="psp", bufs=4, space="PSUM"))
        accp = ctx.enter_context(tc.tile_pool(name="accp", bufs=1, space="PSUM"))

        # ---- load constants (u-chain inputs first: longest dep chain) ----
        xbTa_t = cons.tile([DI + 1, ROWS], bf16)
        nc.sync.dma_start(xbTa_t[:], xbTa_d[:, :])
        wfta_t = cons.tile([DI + 1, DO], bf16)
        nc.sync.dma_start(wfta_t[:], wfta_d[:, :])
        xT_t = cons.tile([DI, N], bf16)
        nc.sync.dma_start(xT_t[:], xT_d[:, :])
        wfsT_t = cons.tile([DI, DO], bf16)
        nc.sync.dma_start(wfsT_t[:], wfsT_d[:, :])
        ws_t = cons.tile([DI, 1], bf16)
        nc.sync.dma_start(ws_t[:], ws_d[:, :])
        wta_t = cons.tile([DI + 1, 1], bf16)
        nc.sync.dma_start(wta_t[:], wta_d[:, :])
        ones_t = cons.tile([1, ROWS], bf16)
        nc.sync.dma_start(ones_t[:], ones_d[:, :])

        # ---- stage 1: small matmuls (all bf16) ----
        # u[i, d] = xb @ WfT.T + bf  (K=65 with ones row folding bf).
        # u gates urep (DRAM round-trip) which gates every build add.
        u_ps = psp.tile([ROWS, DO], f32, tag="pre")
        nc.tensor.matmul(u_ps[:], xbTa_t[:], wfta_t[:], start=True, stop=True)
        u_sb = cons.tile([ROWS, DO], bf16)
        nc.vector.tensor_copy(u_sb[:], u_ps[:])
        u_dram = nc.dram_tensor("u_stage", [F_FULL], bf16).ap()
        nc.sync.dma_start(out=u_dram.rearrange("(i d) -> i d", i=ROWS),
                          in_=u_sb[:, :])
        urep = cons.tile([128, F_FULL], bf16)
        for g in range(4):
            sl = slice(2048 * g, 2048 * (g + 1))
            src = u_dram[sl]
            bsrc = bass.AP(tensor=src.tensor, offset=src.offset,
                           ap=[[0, 128]] + [list(d) for d in src.ap])
            nc.gpsimd.dma_start(out=urep[:, sl], in_=bsrc)

        # ys_jp[j_local, 64*c + d] = ys[128*c + j_local, d]
        ys_jp = cons.tile([128, NCHUNK * DO], bf16)
        for c in range(NCHUNK):
            ysp = psp.tile([128, DO], f32, tag="pre", name=f"ysp{c}")
            nc.tensor.matmul(ysp[:], xT_t[:, 128 * c:128 * (c + 1)], wfsT_t[:],
                             start=True, stop=True)
            nc.vector.tensor_copy(ys_jp[:, DO * c:DO * (c + 1)], ysp[:])

        adjmT_t = cons.tile([128, NCHUNK * ROWS], bf16)
        nc.gpsimd.dma_start(
            adjmT_t[:, :].rearrange("p (c i) -> p c i", c=NCHUNK),
            adjmT_d[:, :].rearrange("(c p) i -> p c i", p=128))
        ident_t = cons.tile([128, 128], bf16)
        nc.gpsimd.dma_start(ident_t[:], ident_d[:, :])

        # a_tgt + bw  [1, ROWS]
        atgt_ps = psp.tile([1, ROWS], f32, tag="pre")
        nc.tensor.matmul(atgt_ps[:], wta_t[:], xbTa_t[:], start=True, stop=True)
        atgt_sb = cons.tile([1, ROWS], bf16)
        nc.scalar.copy(atgt_sb[:], atgt_ps[:])
        # a_src [1, N]
        asrc_sb = cons.tile([1, N], bf16)
        for h in range(2):
            hs = slice(512 * h, 512 * (h + 1))
            asp = psp.tile([1, 512], f32, tag="pre", name=f"asp{h}")
            nc.tensor.matmul(asp[:], ws_t[:], xT_t[:, hs], start=True, stop=True)
            nc.scalar.copy(asrc_sb[:, hs], asp[:])

        # ---- stages 2-4, software-pipelined ----
        SKEW = 2
        onescol = cons.tile([128, 1], bf16)
        nc.vector.memset(onescol[:], 1.0)
        et_all = cons.tile([128, N], bf16)
        t_acc = accp.tile([128, 2048], f32, tag="acc")
        o_sb = cons.tile([128, 2048], bf16)
        r_tiles = {}

        def emit_build(c):
            r_c = rp.tile([128, F_FULL], bf16, name="r_c")
            r_tiles[c] = r_c
            ys_c = ys_jp[:, DO * c:DO * (c + 1)]
            ys_b = ys_c.rearrange("p d -> p () d").broadcast_to(
                (128, HALF // DO, DO))
            for h in range(2):
                u = 2 * c + h
                sl = slice(HALF * h, HALF * (h + 1))
                z = zp.tile([128, HALF], bf16, name="z")
                zv = z[:, :].rearrange("p (i d) -> p i d", i=HALF // DO)
                uv = urep[:, sl].rearrange("p (i d) -> p i d", i=HALF // DO)
                if c in POOL_ADD_CHUNKS:
                    nc.gpsimd.tensor_tensor(zv, ys_b, uv, ALU.add)
                else:
                    nc.vector.tensor_tensor(zv, ys_b, uv, ALU.add)
                if u in RELU_ACT_UNITS:
                    nc.scalar.activation(r_c[:, sl], z[:], AF.Relu)
                else:
                    nc.vector.tensor_scalar_max(r_c[:, sl], z[:], 0.0)

        def emit_softmax():
            # transposed: e^T[j, i] = exp(asrc[j] + atgt[i] + mask[j, i])
            for c in range(NCHUNK):
                cs = slice(128 * c, 128 * (c + 1))
                aph = psp.tile([128, ROWS], f32, tag="pre", name=f"aph{c}")
                nc.tensor.matmul(aph[:], asrc_sb[:, cs], ones_t[:],
                                 start=True, stop=False, skip_group_check=True)
                nc.tensor.matmul(aph[:], ones_t[:], atgt_sb[:],
                                 start=False, stop=False, skip_group_check=True)
                nc.tensor.matmul(aph[:], ident_t[:], adjmT_t[:, cs],
                                 start=False, stop=True, skip_group_check=True)
                nc.scalar.activation(et_all[:, cs], aph[:], AF.Exp)
            # row sums s (per target i), shipped to host
            s_sb = cons.tile([1, N], f32)
            for h in range(2):
                hs = slice(512 * h, 512 * (h + 1))
                s_ps = psp.tile([1, 512], f32, tag="pre", name=f"s_ps{h}")
                nc.tensor.matmul(s_ps[:], onescol[:], et_all[:, hs],
                                 start=True, stop=True, skip_group_check=True)
                nc.scalar.copy(s_sb[:, hs], s_ps[:])
            nc.sync.dma_start(s_d[:, :], s_sb[:, :])

        def emit_reduce(c, last=False):
            r_c = r_tiles.pop(c)
            if not last:
                for b in range(4):
                    for n2 in range(4):
                        nc.tensor.matmul(
                            t_acc[32 * b:32 * (b + 1), 512 * n2:512 * (n2 + 1)],
                            et_all[:, 128 * c + 32 * b:128 * c + 32 * (b + 1)],
                            r_c[:, 2048 * b + 512 * n2:2048 * b + 512 * (n2 + 1)],
                            start=(c == 0), stop=False,
                            skip_group_check=True,
                            tile_position=(0, 32 * b),
                        )
            else:
                # n2-major: each t_acc 512-col slice completes early so its
                # evac + output DMA overlap the remaining matmuls
                for n2 in range(4):
                    for b in range(4):
                        nc.tensor.matmul(
                            t_acc[32 * b:32 * (b + 1), 512 * n2:512 * (n2 + 1)],
                            et_all[:, 128 * c + 32 * b:128 * c + 32 * (b + 1)],
                            r_c[:, 2048 * b + 512 * n2:2048 * b + 512 * (n2 + 1)],
                            start=False, stop=True,
                            skip_group_check=True,
                            tile_position=(0, 32 * b),
                        )
                    sl = slice(512 * n2, 512 * (n2 + 1))
                    if n2 % 2 == 0:
                        nc.scalar.activation(o_sb[:, sl], t_acc[:, sl],
                                             AF.Copy, bias=0.0)
                    else:
                        nc.vector.tensor_copy(o_sb[:, sl], t_acc[:, sl])
                    nc.sync.dma_start(o_d[:, sl], o_sb[:, sl])

        emit_softmax()
        for c in range(NCHUNK + SKEW):
            if c < NCHUNK:
                emit_build(c)
            if c >= SKEW:
                emit_reduce(c - SKEW, last=(c == NCHUNK + SKEW - 1))

    nc.compile()
    return nc


def _prep_inputs(x, adj, Wf, bf_, Ww, bw):
    xT = np.ascontiguousarray(x.T).astype(BF)                        # [64, N]
    wfsT = np.ascontiguousarray(Wf[:, :DI].T).astype(BF)             # [64, 64]
    ws = np.ascontiguousarray(Ww[0, :DI].reshape(DI, 1)).astype(BF)  # [64, 1]
    wta = np.concatenate([Ww[0, DI:], bw]).reshape(DI + 1, 1).astype(BF)
    wfta = np.vstack([Wf[:, DI:].T, bf_[None, :]]).astype(BF)        # [65, 64]
    ident = np.eye(128, dtype=BF)
    onesrow = np.ones((1, ROWS), dtype=BF)

    shared = dict(xT=xT, wfsT=wfsT, ws=ws, wta=wta, wfta=wfta,
                  ident=ident, onesrow=onesrow)
    in_maps = []
    for c in range(N_CORES):
        blk = slice(ROWS * c, ROWS * (c + 1))
        xbTa = np.vstack([x[blk].T, np.ones((1, ROWS), np.float32)])
        m = dict(shared)
        m["xbTa"] = np.ascontiguousarray(xbTa).astype(BF)
        m["adjmT"] = np.where(adj[blk].T > 0, 0.0, -30.0).astype(BF)
        in_maps.append(m)
    return in_maps


def postprocess(results):
    p_idx = np.arange(128)
    col0 = (p_idx % 32) * DO
    out = np.empty((N, DO), np.float32)
    for c in range(N_CORES):
        t = results[c]["o"].astype(np.float32)       # [128, 2048] bf16
        s = results[c]["s8"].astype(np.float32).reshape(1, N)[0]
        s = s.reshape(NCHUNK, ROWS).sum(0)
        diag = t[p_idx[:, None], col0[:, None] + np.arange(DO)[None, :]]
        out[ROWS * c:ROWS * (c + 1)] = diag / s[:, None]
    return out


def get_program():
    if "nc" not in _CACHE:
        _CACHE["nc"] = _build_program()
    return _CACHE["nc"]


def kernel(x, adj, Wf, bf, Ww, bw):
    x = np.asarray(x, dtype=np.float32)
    adj = np.asarray(adj, dtype=np.int32)
    Wf = np.asarray(Wf, dtype=np.float32)
    bf_ = np.asarray(bf, dtype=np.float32)
    Ww = np.asarray(Ww, dtype=np.float32)
    bw = np.asarray(bw, dtype=np.float32)
    assert x.shape == (N, DI) and adj.shape == (N, N)

    nc = get_program()
    in_maps = _prep_inputs(x, adj, Wf, bf_, Ww, bw)
    res = run_bass_kernel_spmd(nc, in_maps, core_ids=list(range(N_CORES)))
    return postprocess(res.results)
